# revision 20
# baseline (speedup 1.0000x reference)
"""Trainium2 Bass kernel for the fused soft-logic-gate layer.

Reference computation:
    pa = softmax(wa, axis=1); pb = softmax(wb, axis=1); pt = softmax(wt, axis=0)
    A = pa @ x; B = pb @ x
    out = sum_g pt[g,:,None] * gate_g(A, B)        (16 soft logic gates)

Every gate is affine in {1, A, B, A*B}, so the 16-gate table collapses to
    out = c0 + cA*A + cB*B + cAB*(A*B)
with four per-row coefficient vectors derived from pt.  Folding the softmax
denominators of wa/wb into those coefficients lets the matmuls run on the raw
exp() weights, and factoring
    out = (A + u) * (cAB*B + cA) + w,   u = cB/cAB,  w = c0 - cA*u
leaves one ACT op + two DVE-class ops per tile (the +w alternates between
ACT and DVE to balance the engines).

x, the weight matrices, and the output are staged in DRAM as float16 (the
rel-err budget is 2e-2; fp16 staging costs ~5e-4), halving HBM traffic.
The weights arrive host-transposed so exp() directly produces the stationary
matmul operands — no PE transposes.  Softmax row sums come from tiny N=2
ones-matmuls against the same stationary blocks.  Weight DMAs lead the sync
HWDGE ring so they complete before the x chunks behind them; outputs follow
on the same ring.  A short burst of warm-up matmuls on a memset tile brings
the PE out of the HAM 4/8 clock gate before the real matmuls arrive.

Sharding: batch axis of x split evenly across 8 NeuronCores (data parallel),
weights replicated.
"""

import os
import sys

for _p in ("/opt/trn_rl_repo",):
    if _p not in sys.path and os.path.isdir(_p):
        sys.path.insert(0, _p)

import numpy as np

SIZE = 256
PREV = 256
BATCH = 32768
N_CORES = 8
BSH = BATCH // N_CORES  # per-core batch shard
CH = 1024               # chunk width (0.5 MiB of fp16 per DMA)
NCH = BSH // CH
P = 128
N_WARM = 6              # PE warm-up matmuls (N=512 each, ~2.6 us cold)

# constants blob layout (f32, [128, 262]):
#   [:, 0]         ones column
#   [:16, 1:6]     sign matrix [16, 5] (cols: sum, c0, cA, cB, cAB)
#   [:16, 6:262]   wt [16, 256]
BLOB_W = 262

_CACHE = {}


def _sign_matrix() -> np.ndarray:
    """[16,5] f32 columns: [colsum, c0, cA, cB, cAB] — gate-table
    coefficients of {1, A, B, A*B} preceded by the softmax denominator."""
    S = np.zeros((16, 5), dtype=np.float32)
    S[:, 0] = 1.0
    S[8:16, 1] = 1.0
    for g in (2, 3, 6, 7):
        S[g, 2] += 1.0
    for g in (8, 9, 12, 13):
        S[g, 2] -= 1.0
    for g in (4, 5, 6, 7):
        S[g, 3] += 1.0
    for g in (8, 9, 10, 11):
        S[g, 3] -= 1.0
    for g, v in {1: 1, 2: -1, 4: -1, 6: -2, 7: -1, 8: 1, 9: 2, 11: 1, 13: 1, 14: -1}.items():
        S[g, 4] = v
    return S


def _build_bass():
    import concourse.bacc as bacc
    import concourse.tile as tile
    import concourse.mybir as mybir

    f32 = mybir.dt.float32
    f16 = mybir.dt.float16
    Act = mybir.ActivationFunctionType
    Alu = mybir.AluOpType

    nc = bacc.Bacc(trn_type="TRN2", target_bir_lowering=False, debug=False,
                   num_devices=N_CORES)

    xs_d = nc.dram_tensor("xs", [PREV, BSH], f16, kind="ExternalInput").ap()
    waT_d = nc.dram_tensor("waT", [PREV, SIZE], f16, kind="ExternalInput").ap()
    wbT_d = nc.dram_tensor("wbT", [PREV, SIZE], f16, kind="ExternalInput").ap()
    bl_d = nc.dram_tensor("blob", [P, BLOB_W], f32, kind="ExternalInput").ap()
    out_d = nc.dram_tensor("out", [SIZE, BSH], f16, kind="ExternalOutput").ap()

    # [p, k/m, ...] views for single-DMA transfers
    xs_v = xs_d.rearrange("(k p) b -> p k b", p=P)
    waT_v = waT_d.rearrange("(k p) m -> p k m", p=P)
    wbT_v = wbT_d.rearrange("(k p) m -> p k m", p=P)
    out_v = out_d.rearrange("(m p) b -> p m b", p=P)

    with tile.TileContext(nc) as tc:
        with tc.tile_pool(name="consts", bufs=1) as consts, \
             tc.tile_pool(name="weights", bufs=1) as weights, \
             tc.tile_pool(name="coefs", bufs=1) as coefs, \
             tc.tile_pool(name="xp", bufs=NCH) as xp:

            # weights on the SWDGE queue (gpsimd is otherwise idle); the x
            # chunks split across the two HWDGE rings so both stream in
            # parallel — a single ring only sustains ~150 GB/s end to end.
            waT_sb = consts.tile([P, 2, SIZE], f16)
            nc.gpsimd.dma_start(out=waT_sb[:], in_=waT_v[:])
            wbT_sb = consts.tile([P, 2, SIZE], f16)
            nc.gpsimd.dma_start(out=wbT_sb[:], in_=wbT_v[:])
            blob = consts.tile([P, BLOB_W], f32)
            nc.gpsimd.dma_start(out=blob[:], in_=bl_d[:])
            smat = blob[:16, 1:6]
            wts = blob[:16, 6:262]

            xtiles = []
            for n in range(NCH):
                xt = xp.tile([P, 2, CH], f16, tag="x", name=f"x{n}")
                eng = nc.sync if n % 2 == 0 else nc.scalar
                eng.dma_start(out=xt[:], in_=xs_v[:, :, n * CH:(n + 1) * CH])
                xtiles.append(xt)

            # tiny early Exp on a memset tile forces the ACT table load off
            # the critical path without waiting on any DMA
            dummy = consts.tile([1, 2], f32)
            nc.vector.memset(dummy[:, 0:1], 0.0)
            nc.scalar.activation(out=dummy[:, 1:2], in_=dummy[:, 0:1], func=Act.Exp)

            ones2 = consts.tile([P, 2], f16)
            nc.vector.memset(ones2[:], 1.0)

            # PE warm-up: matmuls on a zeroed tile, gated on nothing, so the
            # HAM clock gate opens before the real matmuls arrive
            warm = consts.tile([P, 5 * P], f16)
            nc.vector.memset(warm[:], 0.0)

            # exp(host-transposed weights) = stationary operands, directly
            eaT = weights.tile([P, 2, SIZE], f16, tag="eaT")
            nc.scalar.activation(out=eaT[:], in_=waT_sb[:], func=Act.Exp)
            ebT = weights.tile([P, 2, SIZE], f16, tag="ebT")
            nc.scalar.activation(out=ebT[:], in_=wbT_sb[:], func=Act.Exp)

            # [128,2] coefficient tiles (m as free dim):
            cA2 = coefs.tile([P, 2], f32, tag="cA2")
            cAB2 = coefs.tile([P, 2], f32, tag="cAB2")
            cU2 = coefs.tile([P, 2], f32, tag="cU2")
            cW2 = coefs.tile([P, 2], f32, tag="cW2")

            # ---- coefficient preprocessing (off the critical path) ----
            # The warm-up/coefficient PSUM pools close before the main loop
            # so psA/psB can double-buffer across all 8 banks; the overlap
            # deps they leave behind resolve before the first chunk needs
            # those banks.
            prep_ctx = tc.tile_pool(name="prep", bufs=1)
            prep = prep_ctx.__enter__()
            with tc.tile_pool(name="warm_ps", bufs=1, space="PSUM") as warm_ps, \
                 tc.tile_pool(name="coef_ps", bufs=1, space="PSUM") as coef_ps:

                wps = warm_ps.tile([P, 512], f32, tag="warm")
                for i in range(N_WARM):
                    nc.tensor.matmul(wps[:], warm[:, 0:P], warm[:, P:5 * P],
                                     start=True, stop=True, skip_group_check=True)

                ept = prep.tile([16, SIZE], f32, tag="ept")
                nc.scalar.activation(out=ept[:], in_=wts, func=Act.Exp)
                # coefficient + row-sum matmul outputs share one PSUM bank
                cms = coef_ps.tile([P, 20], f32, tag="cms")
                cps = cms[:, 0:10]
                sps = cms[:, 10:18]
                for m in range(2):
                    nc.tensor.matmul(cps[:, m * 5:(m + 1) * 5],
                                     ept[:, m * P:(m + 1) * P], smat,
                                     start=True, stop=True)
                cpsv = cps.rearrange("p (m c) -> p c m", m=2)

                # softmax row sums via N=2 ones-matmuls on the same
                # stationary blocks (accumulated over the two k halves);
                # each sum lands as a duplicated column pair.
                for wi, eT in ((0, eaT), (1, ebT)):
                    for m in range(2):
                        col = wi * 2 + m
                        for k in range(2):
                            nc.tensor.matmul(sps[:, 2 * col:2 * col + 2],
                                             eT[:, k, m * P:(m + 1) * P], ones2[:],
                                             start=(k == 0), stop=(k == 1))
                spsv = sps.rearrange("p (c two) -> p c two", two=2)
                sa2 = spsv[:, 0:2, 0]
                sb2 = spsv[:, 2:4, 0]

                rpt2 = prep.tile([P, 2], f32, tag="rpt2")
                nc.vector.reciprocal(out=rpt2[:], in_=cpsv[:, 0, :])
                rcabn = prep.tile([P, 2], f32, tag="rcabn")
                nc.vector.reciprocal(out=rcabn[:], in_=cpsv[:, 4, :])
                ra2 = prep.tile([P, 2], f32, tag="ra2")
                nc.vector.reciprocal(out=ra2[:], in_=sa2)
                rb2 = prep.tile([P, 2], f32, tag="rb2")
                nc.vector.reciprocal(out=rb2[:], in_=sb2)

                # batched [128,2] coefficient chain:
                h2 = prep.tile([P, 2], f32, tag="h2")
                nc.vector.tensor_tensor(out=h2[:], in0=rpt2[:], in1=ra2[:], op=Alu.mult)
                nc.vector.tensor_tensor(out=cA2[:], in0=cpsv[:, 2, :], in1=h2[:], op=Alu.mult)
                g2 = prep.tile([P, 2], f32, tag="g2")
                nc.vector.tensor_tensor(out=g2[:], in0=h2[:], in1=rb2[:], op=Alu.mult)
                nc.vector.tensor_tensor(out=cAB2[:], in0=cpsv[:, 4, :], in1=g2[:], op=Alu.mult)

                # u = cBn * sa / cABn ;  w = c0n/spt - cA*u
                u2a = prep.tile([P, 2], f32, tag="u2a")
                nc.vector.tensor_tensor(out=u2a[:], in0=cpsv[:, 3, :], in1=rcabn[:], op=Alu.mult)
                nc.vector.tensor_tensor(out=cU2[:], in0=u2a[:], in1=sa2, op=Alu.mult)
                c02 = prep.tile([P, 2], f32, tag="c02")
                nc.vector.tensor_tensor(out=c02[:], in0=cpsv[:, 1, :], in1=rpt2[:], op=Alu.mult)
                t2 = prep.tile([P, 2], f32, tag="t2")
                nc.vector.tensor_tensor(out=t2[:], in0=cA2[:], in1=cU2[:], op=Alu.mult)
                nc.vector.tensor_tensor(out=cW2[:], in0=c02[:], in1=t2[:], op=Alu.subtract)

            # ---- main loop ----
            with tc.tile_pool(name="ep", bufs=3) as ep, \
                 tc.tile_pool(name="ob", bufs=NCH) as ob, \
                 tc.tile_pool(name="psA", bufs=2, space="PSUM") as psA, \
                 tc.tile_pool(name="psB", bufs=2, space="PSUM") as psB:
                for n in range(NCH):
                    xk = xtiles[n]
                    o_sb = ob.tile([P, 2, CH], f16, tag="o", name=f"o{n}")
                    last = n == NCH - 1
                    for m in range(2):
                        a_ps = psA.tile([P, CH], f32, tag="A", name=f"A{n}{m}")
                        b_ps = psB.tile([P, CH], f32, tag="B", name=f"B{n}{m}")
                        for ps_t, eT in ((a_ps, eaT), (b_ps, ebT)):
                            for k in range(2):
                                for s in range(CH // 512):
                                    sl = slice(s * 512, (s + 1) * 512)
                                    nc.tensor.matmul(ps_t[:, sl],
                                                     eT[:, k, m * P:(m + 1) * P],
                                                     xk[:, k, sl],
                                                     start=(k == 0), stop=(k == 1))
                        # out = (A + u) * (cAB*B + cA) + w
                        s_sb = ep.tile([P, CH], f32, tag="s", name=f"s{n}{m}")
                        nc.scalar.activation(out=s_sb[:], in_=b_ps[:], func=Act.Identity,
                                             scale=cAB2[:, m:m + 1], bias=cA2[:, m:m + 1])
                        p_sb = ep.tile([P, CH], f32, tag="p", name=f"p{n}{m}")
                        nc.vector.scalar_tensor_tensor(out=p_sb[:], in0=a_ps[:],
                                                       scalar=cU2[:, m:m + 1], in1=s_sb[:],
                                                       op0=Alu.add, op1=Alu.mult)
                        if m == 0:
                            nc.scalar.activation(out=o_sb[:, m, :], in_=p_sb[:],
                                                 func=Act.Identity,
                                                 bias=cW2[:, m:m + 1])
                        else:
                            nc.vector.tensor_scalar_add(o_sb[:, m, :], p_sb[:],
                                                        cW2[:, m:m + 1])
                        if last:
                            # per-m output pieces drain the tail sooner
                            nc.gpsimd.dma_start(
                                out=out_v[:, m:m + 1, n * CH:(n + 1) * CH],
                                in_=o_sb[:, m:m + 1, :])
                    if not last:
                        nc.gpsimd.dma_start(out=out_v[:, :, n * CH:(n + 1) * CH],
                                            in_=o_sb[:])
            prep_ctx.__exit__(None, None, None)

    nc.compile()
    return nc


def _get_nc():
    if "nc" not in _CACHE:
        _CACHE["nc"] = _build_bass()
    return _CACHE["nc"]


def _make_blob(wt: np.ndarray) -> np.ndarray:
    blob = np.zeros((P, BLOB_W), dtype=np.float32)
    blob[:, 0] = 1.0
    blob[:16, 1:6] = _sign_matrix()
    blob[:16, 6:262] = wt
    return blob


def _dev_inputs(x, wa, wb, wt, core=0):
    x16 = np.asarray(x, dtype=np.float16)
    waT = np.ascontiguousarray(np.asarray(wa, dtype=np.float16).T)
    wbT = np.ascontiguousarray(np.asarray(wb, dtype=np.float16).T)
    wt = np.ascontiguousarray(np.asarray(wt, dtype=np.float32))
    return {
        "xs": np.ascontiguousarray(x16[:, core * BSH:(core + 1) * BSH]),
        "waT": waT, "wbT": wbT, "blob": _make_blob(wt),
    }


def _run(x, wa, wb, wt, trace=False, **spmd_kwargs):
    from concourse import bass_utils

    nc = _get_nc()
    x16 = np.asarray(x, dtype=np.float16)
    waT = np.ascontiguousarray(np.asarray(wa, dtype=np.float16).T)
    wbT = np.ascontiguousarray(np.asarray(wb, dtype=np.float16).T)
    wt = np.ascontiguousarray(np.asarray(wt, dtype=np.float32))
    blob = _make_blob(wt)

    in_maps = []
    for c in range(N_CORES):
        in_maps.append({
            "xs": np.ascontiguousarray(x16[:, c * BSH:(c + 1) * BSH]),
            "waT": waT, "wbT": wbT, "blob": blob,
        })
    res = bass_utils.run_bass_kernel_spmd(nc, in_maps, core_ids=list(range(N_CORES)),
                                          trace=trace, **spmd_kwargs)
    out = np.concatenate(
        [np.asarray(res.results[c]["out"], dtype=np.float32) for c in range(N_CORES)],
        axis=1)
    return out, res


def kernel(x, wa, wb, wt):
    out, _ = _run(x, wa, wb, wt, trace=False)
    return out


# revision 28
# speedup vs baseline: 1.0199x; 1.0199x over previous
"""Trainium2 Bass kernel for the fused soft-logic-gate layer.

Reference computation:
    pa = softmax(wa, axis=1); pb = softmax(wb, axis=1); pt = softmax(wt, axis=0)
    A = pa @ x; B = pb @ x
    out = sum_g pt[g,:,None] * gate_g(A, B)        (16 soft logic gates)

Every gate is affine in {1, A, B, A*B}, so the 16-gate table collapses to
    out = c0 + cA*A + cB*B + cAB*(A*B)
with four per-row coefficient vectors derived from pt.  Folding the softmax
denominators of wa/wb into those coefficients lets the matmuls run on the raw
exp() weights, and factoring
    out = (A + u) * (cAB*B + cA) + w,   u = cB/cAB,  w = c0 - cA*u
leaves one ACT op + two DVE-class ops per tile (the +w alternates between
ACT and DVE to balance the engines).

x, the weight matrices, and the output are staged in DRAM as float16 (the
rel-err budget is 2e-2; fp16 staging costs ~5e-4), halving HBM traffic.
The weights arrive host-transposed so exp() directly produces the stationary
matmul operands — no PE transposes.  Softmax row sums come from tiny N=2
ones-matmuls against the same stationary blocks.  Weight DMAs lead the sync
HWDGE ring so they complete before the x chunks behind them; outputs follow
on the same ring.  A short burst of warm-up matmuls on a memset tile brings
the PE out of the HAM 4/8 clock gate before the real matmuls arrive.

Sharding: batch axis of x split evenly across 8 NeuronCores (data parallel),
weights replicated.
"""

import os
import sys

for _p in ("/opt/trn_rl_repo",):
    if _p not in sys.path and os.path.isdir(_p):
        sys.path.insert(0, _p)

import numpy as np

SIZE = 256
PREV = 256
BATCH = 32768
N_CORES = 8
BSH = BATCH // N_CORES  # per-core batch shard
CH = 1024               # chunk width (0.5 MiB of fp16 per DMA)
NCH = BSH // CH
P = 128
N_WARM = 6              # PE warm-up matmuls (N=512 each, ~2.6 us cold)

# constants blob layout (f32, [128, 262]):
#   [:, 0]         ones column
#   [:16, 1:6]     sign matrix [16, 5] (cols: sum, c0, cA, cB, cAB)
#   [:16, 6:262]   wt [16, 256]
BLOB_W = 262

_CACHE = {}


def _sign_matrix() -> np.ndarray:
    """[16,5] f32 columns: [colsum, c0, cA, cB, cAB] — gate-table
    coefficients of {1, A, B, A*B} preceded by the softmax denominator."""
    S = np.zeros((16, 5), dtype=np.float32)
    S[:, 0] = 1.0
    S[8:16, 1] = 1.0
    for g in (2, 3, 6, 7):
        S[g, 2] += 1.0
    for g in (8, 9, 12, 13):
        S[g, 2] -= 1.0
    for g in (4, 5, 6, 7):
        S[g, 3] += 1.0
    for g in (8, 9, 10, 11):
        S[g, 3] -= 1.0
    for g, v in {1: 1, 2: -1, 4: -1, 6: -2, 7: -1, 8: 1, 9: 2, 11: 1, 13: 1, 14: -1}.items():
        S[g, 4] = v
    return S


def _build_bass():
    import concourse.bacc as bacc
    import concourse.tile as tile
    import concourse.mybir as mybir

    f32 = mybir.dt.float32
    f16 = mybir.dt.float16
    Act = mybir.ActivationFunctionType
    Alu = mybir.AluOpType

    nc = bacc.Bacc(trn_type="TRN2", target_bir_lowering=False, debug=False,
                   num_devices=N_CORES)

    xs_d = nc.dram_tensor("xs", [PREV, BSH], f16, kind="ExternalInput").ap()
    wab_d = nc.dram_tensor("wab", [PREV, 2 * SIZE], f16, kind="ExternalInput").ap()
    bl_d = nc.dram_tensor("blob", [P, BLOB_W], f32, kind="ExternalInput").ap()
    out_d = nc.dram_tensor("out", [SIZE, BSH], f16, kind="ExternalOutput").ap()

    # [p, k/m, ...] views for single-DMA transfers
    xs_v = xs_d.rearrange("(k p) b -> p k b", p=P)
    wab_v = wab_d.rearrange("(k p) m -> p k m", p=P)
    out_v = out_d.rearrange("(m p) b -> p m b", p=P)

    with tile.TileContext(nc) as tc:
        with tc.tile_pool(name="consts", bufs=1) as consts, \
             tc.tile_pool(name="weights", bufs=1) as weights, \
             tc.tile_pool(name="coefs", bufs=1) as coefs, \
             tc.tile_pool(name="xp", bufs=NCH) as xp:

            # DMA plan: queues serialize their transfers (~1-2 us completion
            # latency each), so the critical stream gets its own ring with
            # nothing queued ahead of it.
            #   sync ring:   x in two 1 MiB pieces
            #   scalar ring: both weight matrices as one DMA, then outputs
            #   gpsimd:      blob, then the other half of the outputs
            xhalves = []
            for h in range(2):
                xt = xp.tile([P, 2, 2 * CH], f16, tag="x", name=f"x{h}")
                nc.sync.dma_start(out=xt[:], in_=xs_v[:, :, h * 2 * CH:(h + 1) * 2 * CH])
                xhalves.append(xt)

            wab_sb = consts.tile([P, 2, 2 * SIZE], f16)
            nc.scalar.dma_start(out=wab_sb[:], in_=wab_v[:])
            blob = consts.tile([P, BLOB_W], f32)
            nc.gpsimd.dma_start(out=blob[:], in_=bl_d[:])
            smat = blob[:16, 1:6]
            wts = blob[:16, 6:262]

            # tiny early Exp on a memset tile forces the ACT table load off
            # the critical path without waiting on any DMA
            dummy = consts.tile([1, 2], f32)
            nc.vector.memset(dummy[:, 0:1], 0.0)
            nc.scalar.activation(out=dummy[:, 1:2], in_=dummy[:, 0:1], func=Act.Exp)

            # ept ahead of the weight exps in the ACT FIFO — it only needs
            # the (small, early) blob
            ept = consts.tile([16, SIZE], f32, tag="ept")
            nc.scalar.activation(out=ept[:], in_=wts, func=Act.Exp)

            ones2 = consts.tile([P, 2], f16)
            nc.vector.memset(ones2[:], 1.0)

            # PE warm-up: matmuls on a zeroed tile, gated on nothing, so the
            # HAM clock gate opens before the real matmuls arrive
            warm = consts.tile([P, 5 * P], f16)
            nc.vector.memset(warm[:], 0.0)

            # exp(host-transposed weights) = stationary operands, directly
            eaT = weights.tile([P, 2, SIZE], f16, tag="eaT")
            nc.scalar.activation(out=eaT[:], in_=wab_sb[:, :, 0:SIZE], func=Act.Exp)
            ebT = weights.tile([P, 2, SIZE], f16, tag="ebT")
            nc.scalar.activation(out=ebT[:], in_=wab_sb[:, :, SIZE:2 * SIZE], func=Act.Exp)

            # [128,2] coefficient tiles (m as free dim):
            cA2 = coefs.tile([P, 2], f32, tag="cA2")
            cAB2 = coefs.tile([P, 2], f32, tag="cAB2")
            cU2 = coefs.tile([P, 2], f32, tag="cU2")
            cW2 = coefs.tile([P, 2], f32, tag="cW2")

            # ---- coefficient preprocessing (off the critical path) ----
            # The warm-up/coefficient PSUM pools close before the main loop
            # so psA/psB can double-buffer across all 8 banks; the overlap
            # deps they leave behind resolve before the first chunk needs
            # those banks.
            prep_ctx = tc.tile_pool(name="prep", bufs=1)
            prep = prep_ctx.__enter__()
            with tc.tile_pool(name="warm_ps", bufs=1, space="PSUM") as warm_ps, \
                 tc.tile_pool(name="coef_ps", bufs=1, space="PSUM") as coef_ps:

                wps = warm_ps.tile([P, 512], f32, tag="warm")
                for i in range(N_WARM):
                    nc.tensor.matmul(wps[:], warm[:, 0:P], warm[:, P:5 * P],
                                     start=True, stop=True, skip_group_check=True)

                # coefficient + row-sum matmul outputs share one PSUM bank
                cms = coef_ps.tile([P, 20], f32, tag="cms")
                cps = cms[:, 0:10]
                sps = cms[:, 10:18]
                for m in range(2):
                    nc.tensor.matmul(cps[:, m * 5:(m + 1) * 5],
                                     ept[:, m * P:(m + 1) * P], smat,
                                     start=True, stop=True)
                cpsv = cps.rearrange("p (m c) -> p c m", m=2)

                # softmax row sums via N=2 ones-matmuls on the same
                # stationary blocks (accumulated over the two k halves);
                # each sum lands as a duplicated column pair.
                for wi, eT in ((0, eaT), (1, ebT)):
                    for m in range(2):
                        col = wi * 2 + m
                        for k in range(2):
                            nc.tensor.matmul(sps[:, 2 * col:2 * col + 2],
                                             eT[:, k, m * P:(m + 1) * P], ones2[:],
                                             start=(k == 0), stop=(k == 1))
                spsv = sps.rearrange("p (c two) -> p c two", two=2)
                sa2 = spsv[:, 0:2, 0]
                sb2 = spsv[:, 2:4, 0]

                rpt2 = prep.tile([P, 2], f32, tag="rpt2")
                nc.vector.reciprocal(out=rpt2[:], in_=cpsv[:, 0, :])
                rcabn = prep.tile([P, 2], f32, tag="rcabn")
                nc.vector.reciprocal(out=rcabn[:], in_=cpsv[:, 4, :])
                ra2 = prep.tile([P, 2], f32, tag="ra2")
                nc.vector.reciprocal(out=ra2[:], in_=sa2)
                rb2 = prep.tile([P, 2], f32, tag="rb2")
                nc.vector.reciprocal(out=rb2[:], in_=sb2)

                # batched [128,2] coefficient chain:
                h2 = prep.tile([P, 2], f32, tag="h2")
                nc.vector.tensor_tensor(out=h2[:], in0=rpt2[:], in1=ra2[:], op=Alu.mult)
                nc.vector.tensor_tensor(out=cA2[:], in0=cpsv[:, 2, :], in1=h2[:], op=Alu.mult)
                g2 = prep.tile([P, 2], f32, tag="g2")
                nc.vector.tensor_tensor(out=g2[:], in0=h2[:], in1=rb2[:], op=Alu.mult)
                nc.vector.tensor_tensor(out=cAB2[:], in0=cpsv[:, 4, :], in1=g2[:], op=Alu.mult)

                # u = cBn * sa / cABn ;  w = c0n/spt - cA*u
                u2a = prep.tile([P, 2], f32, tag="u2a")
                nc.vector.tensor_tensor(out=u2a[:], in0=cpsv[:, 3, :], in1=rcabn[:], op=Alu.mult)
                nc.vector.tensor_tensor(out=cU2[:], in0=u2a[:], in1=sa2, op=Alu.mult)
                c02 = prep.tile([P, 2], f32, tag="c02")
                nc.vector.tensor_tensor(out=c02[:], in0=cpsv[:, 1, :], in1=rpt2[:], op=Alu.mult)
                t2 = prep.tile([P, 2], f32, tag="t2")
                nc.vector.tensor_tensor(out=t2[:], in0=cA2[:], in1=cU2[:], op=Alu.mult)
                nc.vector.tensor_tensor(out=cW2[:], in0=c02[:], in1=t2[:], op=Alu.subtract)

            # ---- main loop ----
            with tc.tile_pool(name="ep", bufs=3) as ep, \
                 tc.tile_pool(name="ob", bufs=NCH) as ob, \
                 tc.tile_pool(name="psA", bufs=2, space="PSUM") as psA, \
                 tc.tile_pool(name="psB", bufs=2, space="PSUM") as psB:
                for n in range(NCH):
                    xk = xhalves[n // 2]
                    xoff = (n % 2) * CH
                    o_sb = ob.tile([P, 2, CH], f16, tag="o", name=f"o{n}")
                    last = n == NCH - 1
                    for m in range(2):
                        a_ps = psA.tile([P, CH], f32, tag="A", name=f"A{n}{m}")
                        b_ps = psB.tile([P, CH], f32, tag="B", name=f"B{n}{m}")
                        for ps_t, eT in ((a_ps, eaT), (b_ps, ebT)):
                            for k in range(2):
                                for s in range(CH // 512):
                                    sl = slice(xoff + s * 512, xoff + (s + 1) * 512)
                                    ol = slice(s * 512, (s + 1) * 512)
                                    nc.tensor.matmul(ps_t[:, ol],
                                                     eT[:, k, m * P:(m + 1) * P],
                                                     xk[:, k, sl],
                                                     start=(k == 0), stop=(k == 1))
                        # out = (A + u) * (cAB*B + cA) + w
                        s_sb = ep.tile([P, CH], f32, tag="s", name=f"s{n}{m}")
                        nc.scalar.activation(out=s_sb[:], in_=b_ps[:], func=Act.Identity,
                                             scale=cAB2[:, m:m + 1], bias=cA2[:, m:m + 1])
                        p_sb = ep.tile([P, CH], f32, tag="p", name=f"p{n}{m}")
                        nc.vector.scalar_tensor_tensor(out=p_sb[:], in0=a_ps[:],
                                                       scalar=cU2[:, m:m + 1], in1=s_sb[:],
                                                       op0=Alu.add, op1=Alu.mult)
                        if m == 0:
                            nc.scalar.activation(out=o_sb[:, m, :], in_=p_sb[:],
                                                 func=Act.Identity,
                                                 bias=cW2[:, m:m + 1])
                        else:
                            nc.vector.tensor_scalar_add(o_sb[:, m, :], p_sb[:],
                                                        cW2[:, m:m + 1])
                        if last:
                            # per-m output pieces drain the tail on both queues
                            eng = nc.scalar if m == 0 else nc.gpsimd
                            eng.dma_start(
                                out=out_v[:, m:m + 1, n * CH:(n + 1) * CH],
                                in_=o_sb[:, m:m + 1, :])
                    if not last:
                        eng = nc.scalar if n % 2 == 0 else nc.gpsimd
                        eng.dma_start(out=out_v[:, :, n * CH:(n + 1) * CH],
                                      in_=o_sb[:])
            prep_ctx.__exit__(None, None, None)

    nc.compile()
    return nc


def _get_nc():
    if "nc" not in _CACHE:
        _CACHE["nc"] = _build_bass()
    return _CACHE["nc"]


def _make_blob(wt: np.ndarray) -> np.ndarray:
    blob = np.zeros((P, BLOB_W), dtype=np.float32)
    blob[:, 0] = 1.0
    blob[:16, 1:6] = _sign_matrix()
    blob[:16, 6:262] = wt
    return blob


def _make_wab(wa, wb):
    waT = np.asarray(wa, dtype=np.float16).T
    wbT = np.asarray(wb, dtype=np.float16).T
    return np.ascontiguousarray(np.concatenate([waT, wbT], axis=1))


def _dev_inputs(x, wa, wb, wt, core=0):
    x16 = np.asarray(x, dtype=np.float16)
    wt = np.ascontiguousarray(np.asarray(wt, dtype=np.float32))
    return {
        "xs": np.ascontiguousarray(x16[:, core * BSH:(core + 1) * BSH]),
        "wab": _make_wab(wa, wb), "blob": _make_blob(wt),
    }


def _run(x, wa, wb, wt, trace=False, **spmd_kwargs):
    from concourse import bass_utils

    nc = _get_nc()
    x16 = np.asarray(x, dtype=np.float16)
    wab = _make_wab(wa, wb)
    wt = np.ascontiguousarray(np.asarray(wt, dtype=np.float32))
    blob = _make_blob(wt)

    in_maps = []
    for c in range(N_CORES):
        in_maps.append({
            "xs": np.ascontiguousarray(x16[:, c * BSH:(c + 1) * BSH]),
            "wab": wab, "blob": blob,
        })
    res = bass_utils.run_bass_kernel_spmd(nc, in_maps, core_ids=list(range(N_CORES)),
                                          trace=trace, **spmd_kwargs)
    out = np.concatenate(
        [np.asarray(res.results[c]["out"], dtype=np.float32) for c in range(N_CORES)],
        axis=1)
    return out, res


def kernel(x, wa, wb, wt):
    out, _ = _run(x, wa, wb, wt, trace=False)
    return out
